# revision 4
# baseline (speedup 1.0000x reference)
"""AttentionPooling kernel for 8 Trainium2 NeuronCores.

Computation (per graph g): out[g] = sum_i softmax(logits)_i * x_i over nodes i in g,
where logits = tanh(x @ W1 + b1) @ W2 + b2.

Strategy:
- logits are bounded (|logit| <= sum|W2| + |b2| < 17), so exp() is safe without the
  max-subtraction: w_i = e_i / sum(e) with e_i = exp(logit_i). Single pass over x.
- Shard 8192 graphs across 8 cores (1024 each). Per core, 8 "graph blocks" of 128
  graphs; a block's 128 graphs map to the 128 PSUM partitions of an accumulator.
- batch is known on host: node ranges per block are computed on host and the x rows
  are gathered per (core, block) into fixed-size slabs of T_blk*128 rows, so all 8
  cores run one identical program (SPMD).
- Per 128-node subtile: hT = W1.T-blocks @ xT (PE, bf16), tanh+b1 (ACT),
  logit column = thT_half.T @ W2_half (PE), e = exp(logit+b2) (ACT),
  onehot_e[n, g] = (iota==bid)*e (one DVE tensor_scalar), then
  numer[g, 0:256] += onehot_e.T @ [x | 1] (PE, accumulating in PSUM; col 256 is the
  softmax denominator). Block epilogue divides and DMAs out.
"""

import math
import os
import re
from contextlib import ExitStack

import numpy as np
import ml_dtypes

try:
    import concourse.bass as bass
except ImportError:  # fallback if PYTHONPATH lacks the repo
    import sys

    sys.path.insert(0, "/opt/trn_rl_repo")
    import concourse.bass as bass

import bass_rust
import concourse.tile as tile
from concourse import bass_utils, mybir
from concourse.vector_clock import ScopedClock

BF16 = ml_dtypes.bfloat16
F32 = np.float32

N_CORES = 8
N_NODES = 1_000_000
H = 256  # hidden
G = 8192  # num graphs
GPC = G // N_CORES  # graphs per core = 1024
GPB = 128  # graphs per block (= PSUM partitions)
BPC = GPC // GPB  # blocks per core = 8
P = 128  # partitions / nodes per subtile

def _split_sync_waits(nc, maxw: int = 1) -> int:
    """The walrus build in this container rejects instructions carrying more
    than one sync-wait. Hoist extra waits onto NoOps inserted just before the
    instruction (same engine, same order => identical semantics)."""
    cnt = 0
    for f in nc.m.functions:
        for bb in f.blocks:
            insts = bb.instructions
            out = []
            changed = False
            for ins in insts:
                si = ins.sync_info
                if si is not None and len(si.on_wait) > maxw:
                    waits = list(si.on_wait)
                    keep, extra = waits[-maxw:], waits[:-maxw]
                    for w in extra:
                        cnt += 1
                        nop = mybir.InstNoOp(
                            name=f"wsplit-{cnt}",
                            engine=ins.engine,
                            sync_info=mybir.SyncInfo(on_wait=[w], on_update=[]),
                            bass_nofuse=True,
                        )
                        nc.register_instruction(nop, overwrite=True)
                        out.append(nop)
                    ins.sync_info = mybir.SyncInfo(
                        on_wait=keep, on_update=si.on_update
                    )
                    changed = True
                out.append(ins)
            if changed:
                bb.instructions = out
    return cnt


def _build_program(
    T_blk: int,
    max_subtiles: int | None = None,
    do_mlp: bool = True,
    do_logit: bool = True,
    do_exp: bool = True,
    do_onehot: bool = True,
    do_numer: bool = True,
    do_epi: bool = True,
):
    nc = bass.Bass("TRN2", target_bir_lowering=False)
    T_tot = BPC * T_blk
    L = T_tot * P  # node slots per core

    f32 = mybir.dt.float32
    bf16 = mybir.dt.bfloat16

    xt_d = nc.declare_dram_parameter("xt", [H, L], bf16, isOutput=False)
    xn_d = nc.declare_dram_parameter("xn", [L, H + 1], bf16, isOutput=False)
    bc_d = nc.declare_dram_parameter("bc", [P, T_tot], f32, isOutput=False)
    w1a_d = nc.declare_dram_parameter("w1a", [P, H], bf16, isOutput=False)
    w1b_d = nc.declare_dram_parameter("w1b", [P, H], bf16, isOutput=False)
    w2a_d = nc.declare_dram_parameter("w2a", [P, 1], bf16, isOutput=False)
    w2b_d = nc.declare_dram_parameter("w2b", [P, 1], bf16, isOutput=False)
    b1a_d = nc.declare_dram_parameter("b1a", [P, 1], f32, isOutput=False)
    b1b_d = nc.declare_dram_parameter("b1b", [P, 1], f32, isOutput=False)
    b2c_d = nc.declare_dram_parameter("b2c", [P, 1], f32, isOutput=False)
    iota_d = nc.declare_dram_parameter("iota", [P, P], f32, isOutput=False)
    out_d = nc.declare_dram_parameter("out", [GPC, H], f32, isOutput=True)

    Tanh = mybir.ActivationFunctionType.Tanh
    Exp = mybir.ActivationFunctionType.Exp
    EQ = mybir.AluOpType.is_equal
    MUL = mybir.AluOpType.mult
    ADD = mybir.AluOpType.add

    GRP = 8  # subtiles per DMA group (1024 nodes)
    HG = 4  # subtiles per matmul half-group (N=512)
    assert T_tot % GRP == 0

    with tile.TileContext(nc) as tc:
        with ExitStack() as ctx:
            consts = ctx.enter_context(tc.tile_pool(name="consts", bufs=1))
            xpool = ctx.enter_context(tc.tile_pool(name="x", bufs=3))
            thpool = ctx.enter_context(tc.tile_pool(name="th", bufs=4))
            ohpool = ctx.enter_context(tc.tile_pool(name="oh", bufs=6))
            epool = ctx.enter_context(tc.tile_pool(name="e", bufs=4))
            outpool = ctx.enter_context(tc.tile_pool(name="outp", bufs=2))
            ps_ht = ctx.enter_context(
                tc.tile_pool(name="ps_ht", bufs=2, space=bass.MemorySpace.PSUM)
            )
            ps_lg = ctx.enter_context(
                tc.tile_pool(name="ps_lg", bufs=2, space=bass.MemorySpace.PSUM)
            )
            ps_nm = ctx.enter_context(
                tc.tile_pool(name="ps_nm", bufs=2, space=bass.MemorySpace.PSUM)
            )

            # ---- constants (loaded once) ----
            w1a_t = consts.tile([P, H], bf16)
            nc.sync.dma_start(w1a_t[:], w1a_d[:])
            w1b_t = consts.tile([P, H], bf16)
            nc.sync.dma_start(w1b_t[:], w1b_d[:])
            w2a_t = consts.tile([P, 1], bf16)
            nc.sync.dma_start(w2a_t[:], w2a_d[:])
            w2b_t = consts.tile([P, 1], bf16)
            nc.sync.dma_start(w2b_t[:], w2b_d[:])
            b1a_t = consts.tile([P, 1], f32)
            nc.sync.dma_start(b1a_t[:], b1a_d[:])
            b1b_t = consts.tile([P, 1], f32)
            nc.sync.dma_start(b1b_t[:], b1b_d[:])
            b2c_t = consts.tile([P, 1], f32)
            nc.sync.dma_start(b2c_t[:], b2c_d[:])
            iota_t = consts.tile([P, P], f32)
            nc.sync.dma_start(iota_t[:], iota_d[:])
            bc_t = consts.tile([P, T_tot], f32)
            nc.sync.dma_start(bc_t[:], bc_d[:])

            xn_r = xn_d[:].rearrange("(t p) h -> p t h", p=P)  # [P, T_tot, 257]

            numer = None
            xta = xtb = xnt = None
            tha = thb = ecols = None

            n_sub = T_tot if max_subtiles is None else min(max_subtiles, T_tot)
            for j in range(n_sub):
                blk, t_in_blk = divmod(j, T_blk)
                if t_in_blk == 0:
                    numer = ps_nm.tile([P, H + 1], f32, tag="numer")

                if j % GRP == 0:
                    goff = j * P
                    xta = xpool.tile([P, GRP * P], bf16, tag="xta")
                    nc.sync.dma_start(xta[:], xt_d[0:P, goff : goff + GRP * P])
                    xtb = xpool.tile([P, GRP * P], bf16, tag="xtb")
                    nc.sync.dma_start(xtb[:], xt_d[P : 2 * P, goff : goff + GRP * P])
                    xnt = xpool.tile([P, GRP, H + 1], bf16, tag="xnt")
                    nc.sync.dma_start(xnt[:], xn_r[:, j : j + GRP, :])

                gi = j % GRP  # subtile index within DMA group
                if do_mlp and j % HG == 0:
                    # half-group matmuls for subtiles j..j+3 (N=512)
                    s = (gi // HG) * (HG * P)
                    xta_s = xta[:, s : s + HG * P]
                    xtb_s = xtb[:, s : s + HG * P]
                    hta = ps_ht.tile([P, HG * P], f32, tag="hta")
                    nc.tensor.matmul(
                        hta[:], w1a_t[:, 0:P], xta_s, start=True, stop=False,
                        skip_group_check=True,
                    )
                    nc.tensor.matmul(
                        hta[:], w1b_t[:, 0:P], xtb_s, start=False, stop=True,
                        skip_group_check=True,
                    )
                    htb = ps_ht.tile([P, HG * P], f32, tag="htb")
                    nc.tensor.matmul(
                        htb[:], w1a_t[:, P:H], xta_s, start=True, stop=False,
                        skip_group_check=True,
                    )
                    nc.tensor.matmul(
                        htb[:], w1b_t[:, P:H], xtb_s, start=False, stop=True,
                        skip_group_check=True,
                    )
                    tha = thpool.tile([P, HG * P], bf16, tag="tha")
                    nc.scalar.activation(tha[:], hta[:], Tanh, bias=b1a_t[:])
                    thb = thpool.tile([P, HG * P], bf16, tag="thb")
                    nc.scalar.activation(thb[:], htb[:], Tanh, bias=b1b_t[:])
                    # logit columns for the 4 subtiles
                    if do_logit:
                        lg = ps_lg.tile([P, HG], f32, tag="lg")
                        for ii in range(HG):
                            nc.tensor.matmul(
                                lg[:, ii : ii + 1],
                                tha[:, ii * P : (ii + 1) * P],
                                w2a_t[:],
                                start=True, stop=False, skip_group_check=True,
                            )
                            nc.tensor.matmul(
                                lg[:, ii : ii + 1],
                                thb[:, ii * P : (ii + 1) * P],
                                w2b_t[:],
                                start=False, stop=True, skip_group_check=True,
                            )
                        if do_exp:
                            ecols = epool.tile([P, HG], f32, tag="ecols")
                            nc.scalar.activation(ecols[:], lg[:], Exp, bias=b2c_t[:])

                hi = j % HG
                if do_onehot:
                    oh = ohpool.tile([P, P], bf16, tag="oh")
                    if do_mlp:
                        nc.vector.tensor_scalar(
                            oh[:], iota_t[:], bc_t[:, j : j + 1],
                            ecols[:, hi : hi + 1], EQ, MUL,
                        )
                    else:
                        nc.vector.tensor_scalar(
                            oh[:], iota_t[:], bc_t[:, j : j + 1], 1.0, EQ, MUL,
                        )
                if do_numer:
                    nc.tensor.matmul(
                        numer[:],
                        oh[:],
                        xnt[:, gi, :],
                        start=(t_in_blk == 0),
                        stop=(t_in_blk == T_blk - 1),
                        skip_group_check=True,
                    )

                if do_epi and do_numer and t_in_blk == T_blk - 1:
                    # block epilogue: out[g] = numer[g, :256] / numer[g, 256]
                    dn = epool.tile([P, 1], f32, tag="dn")
                    nc.vector.tensor_scalar(
                        dn[:], numer[:, H : H + 1], 1e-30, None, ADD
                    )
                    rec = epool.tile([P, 1], f32, tag="rec")
                    nc.vector.reciprocal(rec[:], dn[:])
                    outt = outpool.tile([P, H], f32, tag="outt")
                    nc.vector.tensor_scalar(
                        outt[:], numer[:, 0:H], rec[:], None, MUL
                    )
                    nc.sync.dma_start(
                        out_d[blk * GPB : (blk + 1) * GPB, :], outt[:]
                    )

    return nc


def _run_warmup():
    """Run a tiny NEFF touching every engine/op first. The first NEFF executed
    in a fresh process has been observed to hang when it contains the full
    pipeline (ACT table staging race?); a small warmup run avoids it."""
    f32 = mybir.dt.float32
    Tanh = mybir.ActivationFunctionType.Tanh
    Exp = mybir.ActivationFunctionType.Exp
    EQ = mybir.AluOpType.is_equal
    MUL = mybir.AluOpType.mult
    nc = bass.Bass("TRN2", target_bir_lowering=False)
    x_d = nc.declare_dram_parameter("x", [P, P], f32, isOutput=False)
    y_d = nc.declare_dram_parameter("y", [P, P], f32, isOutput=True)
    with tile.TileContext(nc) as tc:
        with ExitStack() as ctx:
            pool = ctx.enter_context(tc.tile_pool(name="p", bufs=2))
            ps = ctx.enter_context(
                tc.tile_pool(name="ps", bufs=1, space=bass.MemorySpace.PSUM)
            )
            t = pool.tile([P, P], f32)
            nc.sync.dma_start(t[:], x_d[:])
            acc = ps.tile([P, P], f32)
            nc.tensor.matmul(acc[:], t[:], t[:], start=True, stop=True)
            t2 = pool.tile([P, P], f32)
            nc.scalar.activation(t2[:], acc[:], Tanh, bias=t[:, 0:1])
            t3 = pool.tile([P, P], f32)
            nc.scalar.activation(t3[:], t2[:], Exp, bias=t[:, 0:1])
            t4 = pool.tile([P, P], f32)
            nc.vector.tensor_scalar(t4[:], t3[:], t[:, 0:1], t[:, 1:2], EQ, MUL)
            t5 = pool.tile([P, 1], f32)
            nc.vector.reciprocal(t5[:], t3[:, 0:1])
            nc.vector.tensor_scalar(t4[:, 0:1], t5[:], t5[:], None, MUL)
            nc.sync.dma_start(y_d[:], t4[:])
    _split_sync_waits(nc)
    xw = np.zeros((P, P), np.float32)
    bass_utils.run_bass_kernel_spmd(
        nc, [{"x": xw} for _ in range(N_CORES)], list(range(N_CORES))
    )


def prepare_inputs(x, batch, W1, b1, W2, b2):
    """Host-side segmentation + per-core gather. Returns (T_blk, in_maps)."""
    x = np.asarray(x, dtype=F32)
    batch = np.asarray(batch).astype(np.int64)
    W1 = np.asarray(W1, dtype=F32)
    b1 = np.asarray(b1, dtype=F32)
    W2 = np.asarray(W2, dtype=F32)
    b2 = np.asarray(b2, dtype=F32)
    assert x.shape == (N_NODES, H) and batch.shape == (N_NODES,)

    # ---- host-side segmentation ----
    block_starts = np.searchsorted(batch, np.arange(0, G + 1, GPB)).astype(np.int64)
    cnts = np.diff(block_starts)
    T_blk = max(1, int(math.ceil(cnts.max() / P)))
    T_tot = BPC * T_blk
    L = T_tot * P

    import time as _time

    _tg = _time.time()
    xt_all = []
    xn_all = []
    bc_all = []
    for c in range(N_CORES):
        xn_c = np.zeros((L, H + 1), dtype=BF16)
        xn_c[:, H] = F32(1.0)
        xt_c = np.zeros((H, L), dtype=BF16)
        bc_c = np.full((P, T_tot), -1.0, dtype=F32)
        for b in range(BPC):
            gblk = c * BPC + b
            s = int(block_starts[gblk])
            e = min(s + T_blk * P, N_NODES)
            n = e - s
            if n <= 0:
                continue
            r0 = b * T_blk * P
            seg = x[s:e]
            xn_c[r0 : r0 + n, 0:H] = seg
            xt_c[:, r0 : r0 + n] = seg.T
            vals = np.full(T_blk * P, -1.0, dtype=F32)
            vals[:n] = (batch[s:e] - gblk * GPB).astype(F32)
            bc_c[:, b * T_blk : (b + 1) * T_blk] = vals.reshape(T_blk, P).T
        xt_all.append(xt_c)
        xn_all.append(xn_c)
        bc_all.append(bc_c)
    print(f"[kernel] host gather: {_time.time()-_tg:.1f}s", flush=True)

    consts = {
        "w1a": W1[0:P, :].astype(BF16),
        "w1b": W1[P:H, :].astype(BF16),
        "w2a": W2[0:P, :].astype(BF16),
        "w2b": W2[P:H, :].astype(BF16),
        "b1a": b1[0:P, None].astype(F32),
        "b1b": b1[P:H, None].astype(F32),
        "b2c": np.full((P, 1), b2[0] if b2.ndim else b2, dtype=F32),
        "iota": np.tile(np.arange(P, dtype=F32), (P, 1)),
    }

    in_maps = [
        {"xt": xt_all[c], "xn": xn_all[c], "bc": bc_all[c], **consts}
        for c in range(N_CORES)
    ]
    return T_blk, in_maps


last_results = None  # set by kernel() when KERNEL_TRACE=1; read by test.py


def bench_program(nc, in_maps, iters: int = 12):
    """Time repeated NEFF executions via the axon PJRT path.

    Mirrors bass2jax.run_bass_via_pjrt but keeps the jitted callable and
    device-resident inputs so per-call deltas ≈ RPC overhead + HW exec.
    Returns (list of per-call seconds, results of last call).
    """
    import time as _time

    import jax
    from jax.sharding import Mesh, PartitionSpec
    from jax.experimental.shard_map import shard_map

    from concourse import bass2jax, mybir as _mybir

    bass2jax.install_neuronx_cc_hook()

    partition_name = (
        nc.partition_id_tensor.name if nc.partition_id_tensor else None
    )
    in_names, out_names, out_avals, zero_outs = [], [], [], []
    for alloc in nc.m.functions[0].allocations:
        if not isinstance(alloc, _mybir.MemoryLocationSet):
            continue
        name = alloc.memorylocations[0].name
        if alloc.kind == "ExternalInput":
            if name != partition_name:
                in_names.append(name)
        elif alloc.kind == "ExternalOutput":
            shape = tuple(alloc.tensor_shape)
            dtype = _mybir.dt.np(alloc.dtype)
            out_avals.append(jax.core.ShapedArray(shape, dtype))
            out_names.append(name)
            zero_outs.append(np.zeros(shape, dtype))
    n_params = len(in_names)
    n_outs = len(out_avals)
    in_names_all = in_names + out_names
    if partition_name is not None:
        in_names_all = in_names_all + [partition_name]

    def _body(*args):
        operands = list(args)
        if partition_name is not None:
            operands.append(bass2jax.partition_id_tensor())
        outs = bass2jax._bass_exec_p.bind(
            *operands,
            out_avals=tuple(out_avals),
            in_names=tuple(in_names_all),
            out_names=tuple(out_names),
            lowering_input_output_aliases=(),
            sim_require_finite=True,
            sim_require_nnan=True,
            nc=nc,
        )
        return tuple(outs)

    devices = jax.devices()[:N_CORES]
    mesh = Mesh(np.asarray(devices), ("core",))
    in_specs = (PartitionSpec("core"),) * (n_params + n_outs)
    out_specs = (PartitionSpec("core"),) * n_outs
    sharded = jax.jit(
        shard_map(
            _body, mesh=mesh, in_specs=in_specs, out_specs=out_specs,
            check_rep=False,
        ),
        keep_unused=True,
    )
    from jax.sharding import NamedSharding

    shd = NamedSharding(mesh, PartitionSpec("core"))
    concat_in = [
        jax.device_put(
            np.concatenate([np.asarray(in_maps[c][nm]) for c in range(N_CORES)], 0),
            shd,
        )
        for nm in in_names
    ]
    concat_zeros = [
        jax.device_put(np.zeros((N_CORES * z.shape[0], *z.shape[1:]), z.dtype), shd)
        for z in zero_outs
    ]
    jax.block_until_ready(concat_in)
    jax.block_until_ready(concat_zeros)

    times = []
    outs = None
    for i in range(iters):
        t0 = _time.perf_counter()
        outs = sharded(*concat_in, *concat_zeros)
        jax.block_until_ready(outs)
        times.append(_time.perf_counter() - t0)
    res = [
        {
            nm: np.asarray(outs[i]).reshape(N_CORES, *out_avals[i].shape)[c]
            for i, nm in enumerate(out_names)
        }
        for c in range(N_CORES)
    ]
    return times, res


def kernel(x, batch, num_graphs, W1, b1, W2, b2):
    import time as _time

    global last_results
    ng = int(num_graphs)
    assert ng == G
    T_blk, in_maps = prepare_inputs(x, batch, W1, b1, W2, b2)

    t0 = _time.time()
    nc = _build_program(T_blk)
    _split_sync_waits(nc)
    print(f"[kernel] build+split: {_time.time()-t0:.1f}s (T_blk={T_blk})", flush=True)

    t0 = _time.time()
    _run_warmup()
    print(f"[kernel] warmup run: {_time.time()-t0:.1f}s", flush=True)

    t0 = _time.time()
    res = bass_utils.run_bass_kernel_spmd(nc, in_maps, list(range(N_CORES)))
    print(f"[kernel] main run (compile+upload+exec): {_time.time()-t0:.1f}s", flush=True)

    out = np.concatenate([res.results[c]["out"] for c in range(N_CORES)], axis=0)
    return out.astype(F32)



# revision 5
# speedup vs baseline: 8.8432x; 8.8432x over previous
"""AttentionPooling kernel for 8 Trainium2 NeuronCores.

Computation (per graph g): out[g] = sum_i softmax(logits)_i * x_i over nodes i in g,
where logits = tanh(x @ W1 + b1) @ W2 + b2.

Strategy:
- logits are bounded (|logit| <= sum|W2| + |b2| < 17), so exp() is safe without the
  max-subtraction: w_i = e_i / sum(e) with e_i = exp(logit_i). Single pass over x.
- Shard 8192 graphs across 8 cores (1024 each). Per core, 8 "graph blocks" of 128
  graphs; a block's 128 graphs map to the 128 PSUM partitions of an accumulator.
- batch is known on host: node ranges per block are computed on host and the x rows
  are gathered per (core, block) into fixed-size slabs of T_blk*128 rows, so all 8
  cores run one identical program (SPMD).
- Per 128-node subtile: hT = W1.T-blocks @ xT (PE, bf16), tanh+b1 (ACT),
  logit column = thT_half.T @ W2_half (PE), e = exp(logit+b2) (ACT),
  onehot_e[n, g] = (iota==bid)*e (one DVE tensor_scalar), then
  numer[g, 0:256] += onehot_e.T @ [x | 1] (PE, accumulating in PSUM; col 256 is the
  softmax denominator). Block epilogue divides and DMAs out.
"""

import math
import os
import re
from contextlib import ExitStack

import numpy as np
import ml_dtypes

try:
    import concourse.bass as bass
except ImportError:  # fallback if PYTHONPATH lacks the repo
    import sys

    sys.path.insert(0, "/opt/trn_rl_repo")
    import concourse.bass as bass

import bass_rust
import concourse.tile as tile
from concourse import bass_utils, mybir
from concourse.vector_clock import ScopedClock

BF16 = ml_dtypes.bfloat16
F32 = np.float32

N_CORES = 8
N_NODES = 1_000_000
H = 256  # hidden
G = 8192  # num graphs
GPC = G // N_CORES  # graphs per core = 1024
GPB = 128  # graphs per block (= PSUM partitions)
BPC = GPC // GPB  # blocks per core = 8
P = 128  # partitions / nodes per subtile

def _split_sync_waits(nc, maxw: int = 1) -> int:
    """The walrus build in this container rejects instructions carrying more
    than one sync-wait. Hoist extra waits onto NoOps inserted just before the
    instruction (same engine, same order => identical semantics)."""
    cnt = 0
    for f in nc.m.functions:
        for bb in f.blocks:
            insts = bb.instructions
            out = []
            changed = False
            for ins in insts:
                si = ins.sync_info
                if si is not None and len(si.on_wait) > maxw:
                    waits = list(si.on_wait)
                    keep, extra = waits[-maxw:], waits[:-maxw]
                    for w in extra:
                        cnt += 1
                        nop = mybir.InstNoOp(
                            name=f"wsplit-{cnt}",
                            engine=ins.engine,
                            sync_info=mybir.SyncInfo(on_wait=[w], on_update=[]),
                            bass_nofuse=True,
                        )
                        nc.register_instruction(nop, overwrite=True)
                        out.append(nop)
                    ins.sync_info = mybir.SyncInfo(
                        on_wait=keep, on_update=si.on_update
                    )
                    changed = True
                out.append(ins)
            if changed:
                bb.instructions = out
    return cnt


def _build_program(
    T_blk: int,
    max_subtiles: int | None = None,
    do_mlp: bool = True,
    do_logit: bool = True,
    do_exp: bool = True,
    do_onehot: bool = True,
    do_numer: bool = True,
    do_epi: bool = True,
):
    nc = bass.Bass("TRN2", target_bir_lowering=False)
    T_tot = BPC * T_blk
    L = T_tot * P  # node slots per core

    f32 = mybir.dt.float32
    bf16 = mybir.dt.bfloat16

    xt_d = nc.declare_dram_parameter("xt", [H, L], bf16, isOutput=False)
    xn_d = nc.declare_dram_parameter("xn", [L, H + 1], bf16, isOutput=False)
    bc_d = nc.declare_dram_parameter("bc", [P, T_tot], f32, isOutput=False)
    w1a_d = nc.declare_dram_parameter("w1a", [P, H], bf16, isOutput=False)
    w1b_d = nc.declare_dram_parameter("w1b", [P, H], bf16, isOutput=False)
    w2a_d = nc.declare_dram_parameter("w2a", [P, 1], bf16, isOutput=False)
    w2b_d = nc.declare_dram_parameter("w2b", [P, 1], bf16, isOutput=False)
    b1a_d = nc.declare_dram_parameter("b1a", [P, 1], f32, isOutput=False)
    b1b_d = nc.declare_dram_parameter("b1b", [P, 1], f32, isOutput=False)
    b2c_d = nc.declare_dram_parameter("b2c", [P, 1], f32, isOutput=False)
    iota_d = nc.declare_dram_parameter("iota", [P, P], f32, isOutput=False)
    out_d = nc.declare_dram_parameter("out", [GPC, H], f32, isOutput=True)

    Tanh = mybir.ActivationFunctionType.Tanh
    Exp = mybir.ActivationFunctionType.Exp
    EQ = mybir.AluOpType.is_equal
    MUL = mybir.AluOpType.mult
    ADD = mybir.AluOpType.add

    GRP = 8  # subtiles per DMA group (1024 nodes)
    HG = 4  # subtiles per matmul half-group (N=512)
    assert T_tot % GRP == 0

    with tile.TileContext(nc) as tc:
        with ExitStack() as ctx:
            consts = ctx.enter_context(tc.tile_pool(name="consts", bufs=1))
            xpool = ctx.enter_context(tc.tile_pool(name="x", bufs=3))
            thpool = ctx.enter_context(tc.tile_pool(name="th", bufs=4))
            ohpool = ctx.enter_context(tc.tile_pool(name="oh", bufs=6))
            epool = ctx.enter_context(tc.tile_pool(name="e", bufs=4))
            outpool = ctx.enter_context(tc.tile_pool(name="outp", bufs=2))
            ps_ht = ctx.enter_context(
                tc.tile_pool(name="ps_ht", bufs=2, space=bass.MemorySpace.PSUM)
            )
            ps_lg = ctx.enter_context(
                tc.tile_pool(name="ps_lg", bufs=2, space=bass.MemorySpace.PSUM)
            )
            ps_nm = ctx.enter_context(
                tc.tile_pool(name="ps_nm", bufs=2, space=bass.MemorySpace.PSUM)
            )

            # ---- constants (loaded once) ----
            w1a_t = consts.tile([P, H], bf16)
            nc.sync.dma_start(w1a_t[:], w1a_d[:])
            w1b_t = consts.tile([P, H], bf16)
            nc.sync.dma_start(w1b_t[:], w1b_d[:])
            w2a_t = consts.tile([P, 1], bf16)
            nc.sync.dma_start(w2a_t[:], w2a_d[:])
            w2b_t = consts.tile([P, 1], bf16)
            nc.sync.dma_start(w2b_t[:], w2b_d[:])
            b1a_t = consts.tile([P, 1], f32)
            nc.sync.dma_start(b1a_t[:], b1a_d[:])
            b1b_t = consts.tile([P, 1], f32)
            nc.sync.dma_start(b1b_t[:], b1b_d[:])
            b2c_t = consts.tile([P, 1], f32)
            nc.sync.dma_start(b2c_t[:], b2c_d[:])
            iota_t = consts.tile([P, P], f32)
            nc.sync.dma_start(iota_t[:], iota_d[:])
            bc_t = consts.tile([P, T_tot], f32)
            nc.sync.dma_start(bc_t[:], bc_d[:])

            xn_r = xn_d[:].rearrange("(t p) h -> p t h", p=P)  # [P, T_tot, 257]

            numer = None
            xta = xtb = xnt = None
            tha = thb = ecols = None

            n_sub = T_tot if max_subtiles is None else min(max_subtiles, T_tot)
            for j in range(n_sub):
                blk, t_in_blk = divmod(j, T_blk)
                if t_in_blk == 0:
                    numer = ps_nm.tile([P, H + 1], f32, tag="numer")

                if j % GRP == 0:
                    goff = j * P
                    xta = xpool.tile([P, GRP * P], bf16, tag="xta")
                    nc.sync.dma_start(xta[:], xt_d[0:P, goff : goff + GRP * P])
                    xtb = xpool.tile([P, GRP * P], bf16, tag="xtb")
                    nc.sync.dma_start(xtb[:], xt_d[P : 2 * P, goff : goff + GRP * P])
                    xnt = xpool.tile([P, GRP, H + 1], bf16, tag="xnt")
                    nc.sync.dma_start(xnt[:], xn_r[:, j : j + GRP, :])

                gi = j % GRP  # subtile index within DMA group
                if do_mlp and j % HG == 0:
                    # half-group matmuls for subtiles j..j+3 (N=512)
                    s = (gi // HG) * (HG * P)
                    xta_s = xta[:, s : s + HG * P]
                    xtb_s = xtb[:, s : s + HG * P]
                    hta = ps_ht.tile([P, HG * P], f32, tag="hta")
                    nc.tensor.matmul(
                        hta[:], w1a_t[:, 0:P], xta_s, start=True, stop=False,
                        skip_group_check=True,
                    )
                    nc.tensor.matmul(
                        hta[:], w1b_t[:, 0:P], xtb_s, start=False, stop=True,
                        skip_group_check=True,
                    )
                    htb = ps_ht.tile([P, HG * P], f32, tag="htb")
                    nc.tensor.matmul(
                        htb[:], w1a_t[:, P:H], xta_s, start=True, stop=False,
                        skip_group_check=True,
                    )
                    nc.tensor.matmul(
                        htb[:], w1b_t[:, P:H], xtb_s, start=False, stop=True,
                        skip_group_check=True,
                    )
                    tha = thpool.tile([P, HG * P], bf16, tag="tha")
                    nc.scalar.activation(tha[:], hta[:], Tanh, bias=b1a_t[:])
                    thb = thpool.tile([P, HG * P], bf16, tag="thb")
                    nc.scalar.activation(thb[:], htb[:], Tanh, bias=b1b_t[:])
                    # logit columns for the 4 subtiles
                    if do_logit:
                        lg = ps_lg.tile([P, HG], f32, tag="lg")
                        for ii in range(HG):
                            nc.tensor.matmul(
                                lg[:, ii : ii + 1],
                                tha[:, ii * P : (ii + 1) * P],
                                w2a_t[:],
                                start=True, stop=False, skip_group_check=True,
                            )
                            nc.tensor.matmul(
                                lg[:, ii : ii + 1],
                                thb[:, ii * P : (ii + 1) * P],
                                w2b_t[:],
                                start=False, stop=True, skip_group_check=True,
                            )
                        if do_exp:
                            ecols = epool.tile([P, HG], f32, tag="ecols")
                            nc.scalar.activation(ecols[:], lg[:], Exp, bias=b2c_t[:])

                hi = j % HG
                if do_onehot:
                    oh = ohpool.tile([P, P], bf16, tag="oh")
                    if do_mlp:
                        nc.vector.tensor_scalar(
                            oh[:], iota_t[:], bc_t[:, j : j + 1],
                            ecols[:, hi : hi + 1], EQ, MUL,
                        )
                    else:
                        nc.vector.tensor_scalar(
                            oh[:], iota_t[:], bc_t[:, j : j + 1], 1.0, EQ, MUL,
                        )
                if do_numer:
                    nc.tensor.matmul(
                        numer[:],
                        oh[:],
                        xnt[:, gi, :],
                        start=(t_in_blk == 0),
                        stop=(t_in_blk == T_blk - 1),
                        skip_group_check=True,
                    )

                if do_epi and do_numer and t_in_blk == T_blk - 1:
                    # block epilogue: out[g] = numer[g, :256] / numer[g, 256]
                    dn = epool.tile([P, 1], f32, tag="dn")
                    nc.vector.tensor_scalar(
                        dn[:], numer[:, H : H + 1], 1e-30, None, ADD
                    )
                    rec = epool.tile([P, 1], f32, tag="rec")
                    nc.vector.reciprocal(rec[:], dn[:])
                    outt = outpool.tile([P, H], f32, tag="outt")
                    nc.vector.tensor_scalar(
                        outt[:], numer[:, 0:H], rec[:], None, MUL
                    )
                    nc.sync.dma_start(
                        out_d[blk * GPB : (blk + 1) * GPB, :], outt[:]
                    )

    return nc


def _run_warmup():
    """Run a tiny NEFF touching every engine/op first. The first NEFF executed
    in a fresh process has been observed to hang when it contains the full
    pipeline (ACT table staging race?); a small warmup run avoids it."""
    f32 = mybir.dt.float32
    Tanh = mybir.ActivationFunctionType.Tanh
    Exp = mybir.ActivationFunctionType.Exp
    EQ = mybir.AluOpType.is_equal
    MUL = mybir.AluOpType.mult
    nc = bass.Bass("TRN2", target_bir_lowering=False)
    x_d = nc.declare_dram_parameter("x", [P, P], f32, isOutput=False)
    y_d = nc.declare_dram_parameter("y", [P, P], f32, isOutput=True)
    with tile.TileContext(nc) as tc:
        with ExitStack() as ctx:
            pool = ctx.enter_context(tc.tile_pool(name="p", bufs=2))
            ps = ctx.enter_context(
                tc.tile_pool(name="ps", bufs=1, space=bass.MemorySpace.PSUM)
            )
            t = pool.tile([P, P], f32)
            nc.sync.dma_start(t[:], x_d[:])
            acc = ps.tile([P, P], f32)
            nc.tensor.matmul(acc[:], t[:], t[:], start=True, stop=True)
            t2 = pool.tile([P, P], f32)
            nc.scalar.activation(t2[:], acc[:], Tanh, bias=t[:, 0:1])
            t3 = pool.tile([P, P], f32)
            nc.scalar.activation(t3[:], t2[:], Exp, bias=t[:, 0:1])
            t4 = pool.tile([P, P], f32)
            nc.vector.tensor_scalar(t4[:], t3[:], t[:, 0:1], t[:, 1:2], EQ, MUL)
            t5 = pool.tile([P, 1], f32)
            nc.vector.reciprocal(t5[:], t3[:, 0:1])
            nc.vector.tensor_scalar(t4[:, 0:1], t5[:], t5[:], None, MUL)
            nc.sync.dma_start(y_d[:], t4[:])
    _split_sync_waits(nc)
    xw = np.zeros((P, P), np.float32)
    bass_utils.run_bass_kernel_spmd(
        nc, [{"x": xw} for _ in range(N_CORES)], list(range(N_CORES))
    )


def prepare_inputs(x, batch, W1, b1, W2, b2):
    """Host-side segmentation + per-core gather. Returns (T_blk, in_maps)."""
    x = np.asarray(x, dtype=F32)
    batch = np.asarray(batch).astype(np.int64)
    W1 = np.asarray(W1, dtype=F32)
    b1 = np.asarray(b1, dtype=F32)
    W2 = np.asarray(W2, dtype=F32)
    b2 = np.asarray(b2, dtype=F32)
    assert x.shape == (N_NODES, H) and batch.shape == (N_NODES,)

    # ---- host-side segmentation ----
    block_starts = np.searchsorted(batch, np.arange(0, G + 1, GPB)).astype(np.int64)
    cnts = np.diff(block_starts)
    T_blk = max(1, int(math.ceil(cnts.max() / P)))
    T_tot = BPC * T_blk
    L = T_tot * P

    import time as _time

    _tg = _time.time()
    xt_all = []
    xn_all = []
    bc_all = []
    for c in range(N_CORES):
        xn_c = np.zeros((L, H + 1), dtype=BF16)
        xn_c[:, H] = F32(1.0)
        xt_c = np.zeros((H, L), dtype=BF16)
        bc_c = np.full((P, T_tot), -1.0, dtype=F32)
        for b in range(BPC):
            gblk = c * BPC + b
            s = int(block_starts[gblk])
            e = min(s + T_blk * P, N_NODES)
            n = e - s
            if n <= 0:
                continue
            r0 = b * T_blk * P
            seg = x[s:e]
            xn_c[r0 : r0 + n, 0:H] = seg
            xt_c[:, r0 : r0 + n] = seg.T
            vals = np.full(T_blk * P, -1.0, dtype=F32)
            vals[:n] = (batch[s:e] - gblk * GPB).astype(F32)
            bc_c[:, b * T_blk : (b + 1) * T_blk] = vals.reshape(T_blk, P).T
        xt_all.append(xt_c)
        xn_all.append(xn_c)
        bc_all.append(bc_c)
    print(f"[kernel] host gather: {_time.time()-_tg:.1f}s", flush=True)

    consts = {
        "w1a": W1[0:P, :].astype(BF16),
        "w1b": W1[P:H, :].astype(BF16),
        "w2a": W2[0:P, :].astype(BF16),
        "w2b": W2[P:H, :].astype(BF16),
        "b1a": b1[0:P, None].astype(F32),
        "b1b": b1[P:H, None].astype(F32),
        "b2c": np.full((P, 1), b2[0] if b2.ndim else b2, dtype=F32),
        "iota": np.tile(np.arange(P, dtype=F32), (P, 1)),
    }

    in_maps = [
        {"xt": xt_all[c], "xn": xn_all[c], "bc": bc_all[c], **consts}
        for c in range(N_CORES)
    ]
    return T_blk, in_maps


last_results = None  # set by kernel() when KERNEL_TRACE=1; read by test.py


def bench_program(nc, in_maps, iters: int = 12):
    """Time repeated NEFF executions via the axon PJRT path.

    Mirrors bass2jax.run_bass_via_pjrt but keeps the jitted callable and
    device-resident inputs so per-call deltas ≈ RPC overhead + HW exec.
    Returns (list of per-call seconds, results of last call).
    """
    import time as _time

    import jax
    from jax.sharding import Mesh, PartitionSpec
    from jax.experimental.shard_map import shard_map

    from concourse import bass2jax, mybir as _mybir

    bass2jax.install_neuronx_cc_hook()

    partition_name = (
        nc.partition_id_tensor.name if nc.partition_id_tensor else None
    )
    in_names, out_names, out_avals, zero_outs = [], [], [], []
    for alloc in nc.m.functions[0].allocations:
        if not isinstance(alloc, _mybir.MemoryLocationSet):
            continue
        name = alloc.memorylocations[0].name
        if alloc.kind == "ExternalInput":
            if name != partition_name:
                in_names.append(name)
        elif alloc.kind == "ExternalOutput":
            shape = tuple(alloc.tensor_shape)
            dtype = _mybir.dt.np(alloc.dtype)
            out_avals.append(jax.core.ShapedArray(shape, dtype))
            out_names.append(name)
            zero_outs.append(np.zeros(shape, dtype))
    n_params = len(in_names)
    n_outs = len(out_avals)
    in_names_all = in_names + out_names
    if partition_name is not None:
        in_names_all = in_names_all + [partition_name]

    def _body(*args):
        operands = list(args)
        if partition_name is not None:
            operands.append(bass2jax.partition_id_tensor())
        outs = bass2jax._bass_exec_p.bind(
            *operands,
            out_avals=tuple(out_avals),
            in_names=tuple(in_names_all),
            out_names=tuple(out_names),
            lowering_input_output_aliases=(),
            sim_require_finite=True,
            sim_require_nnan=True,
            nc=nc,
        )
        return tuple(outs)

    devices = jax.devices()[:N_CORES]
    mesh = Mesh(np.asarray(devices), ("core",))
    in_specs = (PartitionSpec("core"),) * (n_params + n_outs)
    out_specs = (PartitionSpec("core"),) * n_outs
    sharded = jax.jit(
        shard_map(
            _body, mesh=mesh, in_specs=in_specs, out_specs=out_specs,
            check_rep=False,
        ),
        keep_unused=True,
    )
    from jax.sharding import NamedSharding

    shd = NamedSharding(mesh, PartitionSpec("core"))
    concat_in = [
        jax.device_put(
            np.concatenate([np.asarray(in_maps[c][nm]) for c in range(N_CORES)], 0),
            shd,
        )
        for nm in in_names
    ]
    concat_zeros = [
        jax.device_put(np.zeros((N_CORES * z.shape[0], *z.shape[1:]), z.dtype), shd)
        for z in zero_outs
    ]
    jax.block_until_ready(concat_in)
    jax.block_until_ready(concat_zeros)

    # warmup (compile + first exec)
    outs = sharded(*concat_in, *concat_zeros)
    jax.block_until_ready(outs)

    def timed_batch(k):
        t0 = _time.perf_counter()
        os_ = [sharded(*concat_in, *concat_zeros) for _ in range(k)]
        jax.block_until_ready(os_)
        return _time.perf_counter() - t0

    times = {}
    for k in (1, 4, 16):
        times[k] = [timed_batch(k) for _ in range(3)]
    res = [
        {
            nm: np.asarray(outs[i]).reshape(N_CORES, *out_avals[i].shape)[c]
            for i, nm in enumerate(out_names)
        }
        for c in range(N_CORES)
    ]
    return times, res


def kernel(x, batch, num_graphs, W1, b1, W2, b2):
    import time as _time

    global last_results
    ng = int(num_graphs)
    assert ng == G
    T_blk, in_maps = prepare_inputs(x, batch, W1, b1, W2, b2)

    t0 = _time.time()
    nc = _build_program(T_blk)
    _split_sync_waits(nc)
    print(f"[kernel] build+split: {_time.time()-t0:.1f}s (T_blk={T_blk})", flush=True)

    t0 = _time.time()
    _run_warmup()
    print(f"[kernel] warmup run: {_time.time()-t0:.1f}s", flush=True)

    t0 = _time.time()
    res = bass_utils.run_bass_kernel_spmd(nc, in_maps, list(range(N_CORES)))
    print(f"[kernel] main run (compile+upload+exec): {_time.time()-t0:.1f}s", flush=True)

    out = np.concatenate([res.results[c]["out"] for c in range(N_CORES)], axis=0)
    return out.astype(F32)



# revision 20
# speedup vs baseline: 38.0697x; 4.3050x over previous
"""AttentionPooling kernel for 8 Trainium2 NeuronCores.

Computation (per graph g): out[g] = sum_i softmax(logits)_i * x_i over nodes i in g,
where logits = tanh(x @ W1 + b1) @ W2 + b2.

Strategy (v2):
- logits are bounded (|logit| <= sum|W2| + |b2| < 17), so exp() is safe without the
  max-subtraction: w_i = e_i / sum(e) with e_i = exp(logit_i). Single pass over x.
- Shard 8192 graphs across 8 cores (1024 each). Per core, 8 "graph blocks" of 128
  graphs; a block's 128 graphs map to the 128 PSUM partitions of an accumulator.
- batch is known on host: node ranges per block are computed on host and the x rows
  are gathered per (core, block) into fixed-size slabs of T_blk*128 rows, so all 8
  cores run one identical program (SPMD).
- W1 and W2 are scaled by 32 on host so their fp8(e4m3) encodings stay in the
  normal range; the ACT affine input (scale=1/32) undoes it exactly.
- Per chunk of 8 subtiles (1024 nodes): h32.T = (32 W1).T @ xT via fp8 DoubleRow
  matmuls (K=256 packed, rhs from a [128,2,L] interleaved x.T layout), one
  N=1024 tanh per hidden half (ACT, bias=b1 half, scale=1/32) emitting fp8 th,
  16 tiny matmuls th_slice.T @ (32 W2 half) accumulate logit columns [128, 8],
  one exp (bias=b2, scale=1/32) -> e columns.
- Per 128-node subtile: onehot_e[n, g] = (iota==bid)*e (one DVE tensor_scalar),
  then numer[g, 0:256] += onehot_e.T @ [x | 1] (PE, accumulating in PSUM; col 256
  is the softmax denominator). Block epilogue divides and DMAs out.
"""

import math
import os
import re
from contextlib import ExitStack

import numpy as np
import ml_dtypes

try:
    import concourse.bass as bass
except ImportError:  # fallback if PYTHONPATH lacks the repo
    import sys

    sys.path.insert(0, "/opt/trn_rl_repo")
    import concourse.bass as bass

import bass_rust
import concourse.tile as tile
from concourse import bass_utils, mybir
from concourse.vector_clock import ScopedClock

BF16 = ml_dtypes.bfloat16
FP8 = ml_dtypes.float8_e4m3
F32 = np.float32

N_CORES = 8
N_NODES = 1_000_000
H = 256  # hidden
G = 8192  # num graphs
GPC = G // N_CORES  # graphs per core = 1024
GPB = 128  # graphs per block (= PSUM partitions)
BPC = GPC // GPB  # blocks per core = 8
P = 128  # partitions / nodes per subtile

CH = 8  # subtiles per compute chunk (1024 nodes)
ECH = 2 * CH  # subtiles per exp batch (2 chunks)
GRP = 16  # subtiles per DMA group
WSCALE = 32.0  # host-side W1/W2 scaling (undone by ACT scale=1/32)

USE_FP8 = os.environ.get("KERNEL_FP8", "1") == "1"  # x + W1 in fp8 (DoubleRow)
USE_FP8_TH = os.environ.get("KERNEL_FP8_TH", "0") == "1"  # th + W2 in fp8


def _split_sync_waits(nc, maxw: int = 1) -> int:
    """The walrus build in this container rejects instructions carrying more
    than one sync-wait. Hoist extra waits onto NoOps inserted just before the
    instruction (same engine, same order => identical semantics)."""
    cnt = 0
    for f in nc.m.functions:
        for bb in f.blocks:
            insts = bb.instructions
            out = []
            changed = False
            for ins in insts:
                si = ins.sync_info
                if si is not None and len(si.on_wait) > maxw:
                    waits = list(si.on_wait)
                    keep, extra = waits[-maxw:], waits[:-maxw]
                    for w in extra:
                        cnt += 1
                        nop = mybir.InstNoOp(
                            name=f"wsplit-{cnt}",
                            engine=ins.engine,
                            sync_info=mybir.SyncInfo(on_wait=[w], on_update=[]),
                            bass_nofuse=True,
                        )
                        nc.register_instruction(nop, overwrite=True)
                        out.append(nop)
                    ins.sync_info = mybir.SyncInfo(
                        on_wait=keep, on_update=si.on_update
                    )
                    changed = True
                out.append(ins)
            if changed:
                bb.instructions = out
    return cnt


def _build_program(
    T_blk: int,
    use_fp8: bool = USE_FP8,
    use_fp8_th: bool = USE_FP8_TH,
    repeats: int = 1,
):
    nc = bass.Bass("TRN2", target_bir_lowering=False)
    T_tot = BPC * T_blk
    L = T_tot * P  # node slots per core
    assert T_tot % GRP == 0 and T_tot % ECH == 0 and GRP % CH == 0

    f32 = mybir.dt.float32
    bf16 = mybir.dt.bfloat16
    fp8 = mybir.dt.float8e4
    xt_dt = fp8 if use_fp8 else bf16
    w_dt = fp8 if use_fp8 else bf16
    th_dt = fp8 if use_fp8_th else bf16
    w2_dt = fp8 if use_fp8_th else bf16

    xt_d = nc.declare_dram_parameter("xt", [P, 2, L], xt_dt, isOutput=False)
    xn_d = nc.declare_dram_parameter("xn", [L, H + 1], bf16, isOutput=False)
    bc_d = nc.declare_dram_parameter("bc", [P, T_tot], f32, isOutput=False)
    w1_d = nc.declare_dram_parameter("w1", [P, 2, H], w_dt, isOutput=False)
    w2a_d = nc.declare_dram_parameter("w2a", [P, 1], w2_dt, isOutput=False)
    w2b_d = nc.declare_dram_parameter("w2b", [P, 1], w2_dt, isOutput=False)
    b1a_d = nc.declare_dram_parameter("b1a", [P, 1], f32, isOutput=False)
    b1b_d = nc.declare_dram_parameter("b1b", [P, 1], f32, isOutput=False)
    b2c_d = nc.declare_dram_parameter("b2c", [P, 1], f32, isOutput=False)
    iota_d = nc.declare_dram_parameter("iota", [P, P], f32, isOutput=False)
    out_d = nc.declare_dram_parameter("out", [GPC, H], f32, isOutput=True)

    Tanh = mybir.ActivationFunctionType.Tanh
    Exp = mybir.ActivationFunctionType.Exp
    EQ = mybir.AluOpType.is_equal
    MUL = mybir.AluOpType.mult
    ADD = mybir.AluOpType.add
    DR = mybir.MatmulPerfMode.DoubleRow if use_fp8 else None
    ISCALE = 1.0 / WSCALE

    with tile.TileContext(nc) as tc:
        with ExitStack() as ctx:
            consts = ctx.enter_context(tc.tile_pool(name="consts", bufs=1))
            xpool = ctx.enter_context(tc.tile_pool(name="x", bufs=3))
            thpool = ctx.enter_context(tc.tile_pool(name="th", bufs=4))
            ohpool = ctx.enter_context(tc.tile_pool(name="oh", bufs=6))
            epool = ctx.enter_context(tc.tile_pool(name="e", bufs=4))
            outpool = ctx.enter_context(tc.tile_pool(name="outp", bufs=2))
            ps_hta = ctx.enter_context(
                tc.tile_pool(name="ps_hta", bufs=1, space=bass.MemorySpace.PSUM)
            )
            ps_htb = ctx.enter_context(
                tc.tile_pool(name="ps_htb", bufs=1, space=bass.MemorySpace.PSUM)
            )
            ps_lg = ctx.enter_context(
                tc.tile_pool(name="ps_lg", bufs=2, space=bass.MemorySpace.PSUM)
            )
            ps_nm = ctx.enter_context(
                tc.tile_pool(name="ps_nm", bufs=2, space=bass.MemorySpace.PSUM)
            )

            # ---- constants (loaded once) ----
            w1_t = consts.tile([P, 2, H], w_dt)
            nc.sync.dma_start(w1_t[:], w1_d[:])
            w2a_t = consts.tile([P, 1], w2_dt)
            nc.sync.dma_start(w2a_t[:], w2a_d[:])
            w2b_t = consts.tile([P, 1], w2_dt)
            nc.sync.dma_start(w2b_t[:], w2b_d[:])
            b1a_t = consts.tile([P, 1], f32)
            nc.sync.dma_start(b1a_t[:], b1a_d[:])
            b1b_t = consts.tile([P, 1], f32)
            nc.sync.dma_start(b1b_t[:], b1b_d[:])
            b2c_t = consts.tile([P, 1], f32)
            nc.sync.dma_start(b2c_t[:], b2c_d[:])
            iota_t = consts.tile([P, P], f32)
            nc.sync.dma_start(iota_t[:], iota_d[:])
            bc_t = consts.tile([P, T_tot], f32)
            nc.sync.dma_start(bc_t[:], bc_d[:])

            xn_r = xn_d[:].rearrange("(t p) h -> p t h", p=P)  # [P, T_tot, 257]

            numer = None
            xtg = xng = None
            xngs = {}  # subtile j -> (group tile, index within group)
            NH = CH * P // 2  # nodes per matmul half-chunk (512)

            for jb_r in range(0, repeats * T_tot, ECH):  # exp batch (2 chunks)
                jb = jb_r % T_tot
                lg = ps_lg.tile([P, ECH], f32, tag="lg")
                for j0 in range(jb, jb + ECH, CH):  # chunk
                    if j0 % GRP == 0:
                        goff = j0 * P
                        xtg = xpool.tile([P, 2, GRP * P], xt_dt, tag="xtg")
                        nc.sync.dma_start(
                            xtg[:], xt_d[:, :, goff : goff + GRP * P]
                        )
                        xng = xpool.tile([P, GRP, H + 1], bf16, tag="xng")
                        nc.sync.dma_start(xng[:], xn_r[:, j0 : j0 + GRP, :])
                        for jj in range(GRP):
                            xngs[j0 + jj] = (xng, jj)

                    coff = (j0 % GRP) * P  # chunk offset within DMA group
                    hta = ps_hta.tile([P, CH * P], f32, tag="hta")
                    htb = ps_htb.tile([P, CH * P], f32, tag="htb")
                    for q in range(2):  # node halves of the chunk
                        rhs = xtg[:, :, coff + q * NH : coff + (q + 1) * NH]
                        if use_fp8:
                            nc.tensor.matmul(
                                hta[:, q * NH : (q + 1) * NH],
                                w1_t[:, :, 0:P],
                                rhs,
                                start=True, stop=True,
                                perf_mode=DR, skip_group_check=True,
                            )
                            nc.tensor.matmul(
                                htb[:, q * NH : (q + 1) * NH],
                                w1_t[:, :, P:H],
                                rhs,
                                start=True, stop=True,
                                perf_mode=DR, skip_group_check=True,
                            )
                        else:
                            for kk in range(2):
                                nc.tensor.matmul(
                                    hta[:, q * NH : (q + 1) * NH],
                                    w1_t[:, kk, 0:P],
                                    rhs[:, kk, :],
                                    start=(kk == 0), stop=(kk == 1),
                                    skip_group_check=True,
                                )
                                nc.tensor.matmul(
                                    htb[:, q * NH : (q + 1) * NH],
                                    w1_t[:, kk, P:H],
                                    rhs[:, kk, :],
                                    start=(kk == 0), stop=(kk == 1),
                                    skip_group_check=True,
                                )
                    tha = thpool.tile([P, CH * P], th_dt, tag="tha")
                    nc.scalar.activation(
                        tha[:], hta[:], Tanh, bias=b1a_t[:], scale=ISCALE
                    )
                    thb = thpool.tile([P, CH * P], th_dt, tag="thb")
                    nc.scalar.activation(
                        thb[:], htb[:], Tanh, bias=b1b_t[:], scale=ISCALE
                    )
                    lo = j0 - jb  # this chunk's column base in lg
                    for s in range(CH):
                        nc.tensor.matmul(
                            lg[:, lo + s : lo + s + 1],
                            tha[:, s * P : (s + 1) * P],
                            w2a_t[:],
                            start=True, stop=False, skip_group_check=True,
                        )
                        nc.tensor.matmul(
                            lg[:, lo + s : lo + s + 1],
                            thb[:, s * P : (s + 1) * P],
                            w2b_t[:],
                            start=False, stop=True, skip_group_check=True,
                        )
                ecols = epool.tile([P, ECH], f32, tag="ecols")
                nc.scalar.activation(
                    ecols[:], lg[:], Exp, bias=b2c_t[:], scale=ISCALE
                )

                for s in range(ECH):  # per-subtile: onehot + numer + epilogue
                    j = jb + s
                    blk, t_in_blk = divmod(j, T_blk)
                    if t_in_blk == 0:
                        numer = ps_nm.tile([P, H + 1], f32, tag="numer")
                    oh = ohpool.tile([P, P], bf16, tag="oh")
                    nc.vector.tensor_scalar(
                        oh[:], iota_t[:], bc_t[:, j : j + 1],
                        ecols[:, s : s + 1], EQ, MUL,
                    )
                    xng_j, jj = xngs.pop(j)
                    nc.tensor.matmul(
                        numer[:],
                        oh[:],
                        xng_j[:, jj, :],
                        start=(t_in_blk == 0),
                        stop=(t_in_blk == T_blk - 1),
                        skip_group_check=True,
                    )

                    if t_in_blk == T_blk - 1:
                        # block epilogue: out[g] = numer[g,:256] / numer[g,256]
                        dn = epool.tile([P, 1], f32, tag="dn")
                        nc.vector.tensor_scalar(
                            dn[:], numer[:, H : H + 1], 1e-30, None, ADD
                        )
                        rec = epool.tile([P, 1], f32, tag="rec")
                        nc.vector.reciprocal(rec[:], dn[:])
                        outt = outpool.tile([P, H], f32, tag="outt")
                        nc.vector.tensor_scalar(
                            outt[:], numer[:, 0:H], rec[:], None, MUL
                        )
                        nc.sync.dma_start(
                            out_d[blk * GPB : (blk + 1) * GPB, :], outt[:]
                        )

    return nc


def _run_warmup():
    """Run a tiny NEFF touching every engine/op first. The first NEFF executed
    in a fresh process has been observed to hang when it contains the full
    pipeline (ACT table staging race?); a small warmup run avoids it."""
    f32 = mybir.dt.float32
    Tanh = mybir.ActivationFunctionType.Tanh
    Exp = mybir.ActivationFunctionType.Exp
    EQ = mybir.AluOpType.is_equal
    MUL = mybir.AluOpType.mult
    nc = bass.Bass("TRN2", target_bir_lowering=False)
    x_d = nc.declare_dram_parameter("x", [P, P], f32, isOutput=False)
    y_d = nc.declare_dram_parameter("y", [P, P], f32, isOutput=True)
    with tile.TileContext(nc) as tc:
        with ExitStack() as ctx:
            pool = ctx.enter_context(tc.tile_pool(name="p", bufs=2))
            ps = ctx.enter_context(
                tc.tile_pool(name="ps", bufs=1, space=bass.MemorySpace.PSUM)
            )
            t = pool.tile([P, P], f32)
            nc.sync.dma_start(t[:], x_d[:])
            acc = ps.tile([P, P], f32)
            nc.tensor.matmul(acc[:], t[:], t[:], start=True, stop=True)
            t2 = pool.tile([P, P], f32)
            nc.scalar.activation(t2[:], acc[:], Tanh, bias=t[:, 0:1])
            t3 = pool.tile([P, P], f32)
            nc.scalar.activation(t3[:], t2[:], Exp, bias=t[:, 0:1])
            t4 = pool.tile([P, P], f32)
            nc.vector.tensor_scalar(t4[:], t3[:], t[:, 0:1], t[:, 1:2], EQ, MUL)
            t5 = pool.tile([P, 1], f32)
            nc.vector.reciprocal(t5[:], t3[:, 0:1])
            nc.vector.tensor_scalar(t4[:, 0:1], t5[:], t5[:], None, MUL)
            nc.sync.dma_start(y_d[:], t4[:])
    _split_sync_waits(nc)
    xw = np.zeros((P, P), np.float32)
    bass_utils.run_bass_kernel_spmd(
        nc, [{"x": xw} for _ in range(N_CORES)], list(range(N_CORES))
    )


def prepare_inputs(
    x, batch, W1, b1, W2, b2,
    use_fp8: bool = USE_FP8, use_fp8_th: bool = USE_FP8_TH,
):
    """Host-side segmentation + per-core gather. Returns (T_blk, in_maps)."""
    x = np.asarray(x, dtype=F32)
    batch = np.asarray(batch).astype(np.int64)
    W1 = np.asarray(W1, dtype=F32)
    b1 = np.asarray(b1, dtype=F32)
    W2 = np.asarray(W2, dtype=F32)
    b2 = np.asarray(b2, dtype=F32)
    assert x.shape == (N_NODES, H) and batch.shape == (N_NODES,)
    xt_np = FP8 if use_fp8 else BF16
    w_np = FP8 if use_fp8 else BF16
    w2_np = FP8 if use_fp8_th else BF16

    # ---- host-side segmentation ----
    block_starts = np.searchsorted(batch, np.arange(0, G + 1, GPB)).astype(np.int64)
    cnts = np.diff(block_starts)
    T_blk = max(1, int(math.ceil(cnts.max() / P)))
    # pad so T_tot is divisible by GRP (and CH)
    lcm = GRP * CH // math.gcd(GRP, CH)
    q = lcm // math.gcd(BPC, lcm)
    T_blk = int(math.ceil(T_blk / q) * q)
    T_tot = BPC * T_blk
    L = T_tot * P

    import time as _time

    _tg = _time.time()
    xt_all = []
    xn_all = []
    bc_all = []
    for c in range(N_CORES):
        xn_c = np.zeros((L, H + 1), dtype=BF16)
        xn_c[:, H] = F32(1.0)
        xt_c = np.zeros((2, P, L), dtype=xt_np)  # [khalf, p, node]
        bc_c = np.full((P, T_tot), -1.0, dtype=F32)
        for b in range(BPC):
            gblk = c * BPC + b
            s = int(block_starts[gblk])
            e = min(s + T_blk * P, N_NODES)
            n = e - s
            if n <= 0:
                continue
            r0 = b * T_blk * P
            seg = x[s:e]
            xn_c[r0 : r0 + n, 0:H] = seg
            segT = np.ascontiguousarray(seg.T).astype(xt_np)
            xt_c[0, :, r0 : r0 + n] = segT[0:P]
            xt_c[1, :, r0 : r0 + n] = segT[P:H]
            vals = np.full(T_blk * P, -1.0, dtype=F32)
            vals[:n] = (batch[s:e] - gblk * GPB).astype(F32)
            bc_c[:, b * T_blk : (b + 1) * T_blk] = vals.reshape(T_blk, P).T
        xt_all.append(np.ascontiguousarray(xt_c.transpose(1, 0, 2)))  # [p, 2, L]
        xn_all.append(xn_c)
        bc_all.append(bc_c)
    print(f"[kernel] host gather: {_time.time()-_tg:.1f}s", flush=True)

    w1s = (W1 * WSCALE).astype(w_np)  # [256, 256] scaled
    w1_dr = np.empty((P, 2, H), dtype=w_np)
    w1_dr[:, 0, :] = w1s[0:P, :]
    w1_dr[:, 1, :] = w1s[P:H, :]
    w2s = (W2 * WSCALE).astype(w2_np)
    consts = {
        "w1": w1_dr,
        "w2a": w2s[0:P, :],
        "w2b": w2s[P:H, :],
        "b1a": b1[0:P, None].astype(F32),
        "b1b": b1[P:H, None].astype(F32),
        "b2c": np.full((P, 1), b2[0] if b2.ndim else b2, dtype=F32),
        "iota": np.tile(np.arange(P, dtype=F32), (P, 1)),
    }

    in_maps = [
        {"xt": xt_all[c], "xn": xn_all[c], "bc": bc_all[c], **consts}
        for c in range(N_CORES)
    ]
    return T_blk, in_maps


def bench_program(nc, in_maps, iters: int = 12):
    """Time repeated NEFF executions via the axon PJRT path.

    Mirrors bass2jax.run_bass_via_pjrt but keeps the jitted callable and
    device-resident inputs so per-call deltas ≈ RPC overhead + HW exec.
    Returns (dict of batch-size -> per-batch seconds, results of warm call).
    """
    import time as _time

    import jax
    from jax.sharding import Mesh, PartitionSpec
    from jax.experimental.shard_map import shard_map

    from concourse import bass2jax, mybir as _mybir

    bass2jax.install_neuronx_cc_hook()

    partition_name = (
        nc.partition_id_tensor.name if nc.partition_id_tensor else None
    )
    in_names, out_names, out_avals, zero_outs = [], [], [], []
    for alloc in nc.m.functions[0].allocations:
        if not isinstance(alloc, _mybir.MemoryLocationSet):
            continue
        name = alloc.memorylocations[0].name
        if alloc.kind == "ExternalInput":
            if name != partition_name:
                in_names.append(name)
        elif alloc.kind == "ExternalOutput":
            shape = tuple(alloc.tensor_shape)
            dtype = _mybir.dt.np(alloc.dtype)
            out_avals.append(jax.core.ShapedArray(shape, dtype))
            out_names.append(name)
            zero_outs.append(np.zeros(shape, dtype))
    n_params = len(in_names)
    n_outs = len(out_avals)
    in_names_all = in_names + out_names
    if partition_name is not None:
        in_names_all = in_names_all + [partition_name]

    def _body(*args):
        operands = list(args)
        if partition_name is not None:
            operands.append(bass2jax.partition_id_tensor())
        outs = bass2jax._bass_exec_p.bind(
            *operands,
            out_avals=tuple(out_avals),
            in_names=tuple(in_names_all),
            out_names=tuple(out_names),
            lowering_input_output_aliases=(),
            sim_require_finite=True,
            sim_require_nnan=True,
            nc=nc,
        )
        return tuple(outs)

    devices = jax.devices()[:N_CORES]
    mesh = Mesh(np.asarray(devices), ("core",))
    in_specs = (PartitionSpec("core"),) * (n_params + n_outs)
    out_specs = (PartitionSpec("core"),) * n_outs
    sharded = jax.jit(
        shard_map(
            _body, mesh=mesh, in_specs=in_specs, out_specs=out_specs,
            check_rep=False,
        ),
        keep_unused=True,
    )
    from jax.sharding import NamedSharding

    shd = NamedSharding(mesh, PartitionSpec("core"))
    concat_in = [
        jax.device_put(
            np.concatenate([np.asarray(in_maps[c][nm]) for c in range(N_CORES)], 0),
            shd,
        )
        for nm in in_names
    ]
    concat_zeros = [
        jax.device_put(np.zeros((N_CORES * z.shape[0], *z.shape[1:]), z.dtype), shd)
        for z in zero_outs
    ]
    jax.block_until_ready(concat_in)
    jax.block_until_ready(concat_zeros)

    # warmup (compile + first exec)
    outs = sharded(*concat_in, *concat_zeros)
    jax.block_until_ready(outs)

    def timed_batch(k):
        t0 = _time.perf_counter()
        os_ = [sharded(*concat_in, *concat_zeros) for _ in range(k)]
        jax.block_until_ready(os_)
        return _time.perf_counter() - t0

    times = {}
    for k in (2, 16, 64):
        times[k] = [timed_batch(k) for _ in range(3)]
    res = [
        {
            nm: np.asarray(outs[i]).reshape(N_CORES, *out_avals[i].shape)[c]
            for i, nm in enumerate(out_names)
        }
        for c in range(N_CORES)
    ]
    return times, res


last_results = None


def kernel(x, batch, num_graphs, W1, b1, W2, b2):
    import time as _time

    ng = int(num_graphs)
    assert ng == G
    T_blk, in_maps = prepare_inputs(x, batch, W1, b1, W2, b2)

    t0 = _time.time()
    nc = _build_program(T_blk)
    _split_sync_waits(nc)
    print(f"[kernel] build+split: {_time.time()-t0:.1f}s (T_blk={T_blk})", flush=True)

    t0 = _time.time()
    _run_warmup()
    print(f"[kernel] warmup run: {_time.time()-t0:.1f}s", flush=True)

    t0 = _time.time()
    res = bass_utils.run_bass_kernel_spmd(nc, in_maps, list(range(N_CORES)))
    print(f"[kernel] main run (compile+upload+exec): {_time.time()-t0:.1f}s", flush=True)

    out = np.concatenate([res.results[c]["out"] for c in range(N_CORES)], axis=0)
    return out.astype(F32)
